# revision 9
# baseline (speedup 1.0000x reference)
"""Bottleneck adapter (LayerNorm -> down-proj -> GELU -> up-proj -> residual)
as a Bass/Tile kernel for Trainium2, data-parallel over 8 NeuronCores.

Math (per token t, d_model D=2048, rank R=32):
    mu    = mean(x_t);  var = mean(x_t^2) - mu^2;  rstd = 1/sqrt(var+eps)
    ln    = (x_t - mu) * rstd * gamma + beta
    down  = ln @ w_down + b_down
          = rstd * (x_t @ W - mu * S) + b2          # W = gamma[:,None]*w_down
                                                    # S = colsum(W), b2 = beta@w_down + b_down
    out_t = x_t + gelu(down) @ w_up + b_up

The down matmul therefore runs on RAW x (PE-transposed), and the LayerNorm
is applied as a tiny [R, 128] correction afterwards. b_up is folded into the
up matmul as a 33rd contraction row against a ones-row of the lhsT.
"""

import numpy as np

import concourse.bacc as bacc
import concourse.bass as bass
import concourse.tile as tile
from concourse import mybir

F32 = mybir.dt.float32
AF = mybir.ActivationFunctionType
ALU = mybir.AluOpType

D = 2048          # d_model
R = 32            # adapter rank
N_CORES = 8
TOK_TOTAL = 4 * 4096
TOK_PER_CORE = TOK_TOTAL // N_CORES   # 2048
P = 128           # partitions / tokens per tile
N_TILES = TOK_PER_CORE // P           # 16
N_CHUNK = D // P                      # 16 chunks of d per tile
LN_EPS = 1e-5
UP_N = 512        # free-dim per up matmul (one PSUM bank)
N_UP = D // UP_N  # 4


def build_program(reps=1):
    """reps>1 repeats the whole computation in one NEFF — used only by the
    timing harness (wall-clock slope over reps isolates on-device time)."""
    nc = bacc.Bacc(
        "TRN2",
        target_bir_lowering=False,
        debug=False,
        num_devices=N_CORES,
    )

    x_d = nc.dram_tensor("x", [TOK_PER_CORE, D], F32, kind="ExternalInput").ap()
    w_d = nc.dram_tensor("wc", [P, N_CHUNK, R], F32, kind="ExternalInput").ap()
    wu_d = nc.dram_tensor("wu", [R + 1, D], F32, kind="ExternalInput").ap()
    s_d = nc.dram_tensor("s_col", [R, 1], F32, kind="ExternalInput").ap()
    b2_d = nc.dram_tensor("b2_col", [R, 1], F32, kind="ExternalInput").ap()
    e_d = nc.dram_tensor("e_mat", [2, 2 * R], F32, kind="ExternalInput").ap()
    id_d = nc.dram_tensor("ident", [P, P], F32, kind="ExternalInput").ap()
    out_d = nc.dram_tensor("out", [TOK_PER_CORE, D], F32, kind="ExternalOutput").ap()

    with tile.TileContext(nc) as tc:
        with (
            tc.tile_pool(name="consts", bufs=1) as cpool,
            tc.tile_pool(name="xin", bufs=3) as xpool,
            tc.tile_pool(name="xt", bufs=2) as xtpool,
            tc.tile_pool(name="outs", bufs=2) as opool,
            tc.tile_pool(name="small", bufs=2) as spool,
            tc.tile_pool(name="ps_xt", bufs=2, space="PSUM") as ps_xt,
            tc.tile_pool(name="ps_dn", bufs=2, space="PSUM") as ps_dn,
            tc.tile_pool(name="ps_bc", bufs=1, space="PSUM") as ps_bc,
            tc.tile_pool(name="ps_up", bufs=2, space="PSUM") as ps_up,
        ):
            # ---- one-time constant loads ----
            w_sb = cpool.tile([P, N_CHUNK, R], F32)       # W chunks, [p, c, r]
            nc.sync.dma_start(w_sb[:], w_d[:])
            wu_sb = cpool.tile([R + 1, D], F32)           # [w_up; b_up]
            nc.sync.dma_start(wu_sb[:], wu_d[:])
            s_sb = cpool.tile([R, 1], F32)                # colsum(W)
            nc.sync.dma_start(s_sb[:], s_d[:])
            b2_sb = cpool.tile([R, 1], F32)               # beta@w_down + b_down
            nc.sync.dma_start(b2_sb[:], b2_d[:])
            e_sb = cpool.tile([2, 2 * R], F32)            # broadcast selector
            nc.sync.dma_start(e_sb[:], e_d[:])
            id_sb = cpool.tile([P, P], F32)               # identity for PE transpose
            nc.sync.dma_start(id_sb[:], id_d[:])
            eps_sb = cpool.tile([P, 1], F32)              # LN eps as bias AP
            nc.vector.memset(eps_sb[:], LN_EPS)

            for i in range(N_TILES * reps):
                i = i % N_TILES
                tok = slice(i * P, (i + 1) * P)

                # ---- load x tile [128 tok, 2048 d] ----
                x_t = xpool.tile([P, D], F32)
                nc.sync.dma_start(x_t[:], x_d[tok, :])

                # ---- LN stats: mean/var in one DVE pass ----
                st6 = spool.tile([P, 4, 6], F32)
                for g in range(4):
                    nc.vector.bn_stats(st6[:, g, :], x_t[:, g * 512:(g + 1) * 512])
                mv = spool.tile([P, 2], F32)
                nc.vector.bn_aggr(mv[:], st6[:].rearrange("p g d -> p (g d)"))

                # rstd = 1/sqrt(var+eps); m_rs = mean*rstd  -> rm [128, 2]
                std_t = spool.tile([P, 1], F32)
                nc.scalar.activation(std_t[:], mv[:, 1:2], AF.Sqrt, bias=eps_sb[:])
                rm = spool.tile([P, 2], F32)
                nc.vector.reciprocal(rm[:, 0:1], std_t[:])
                nc.vector.tensor_mul(rm[:, 1:2], mv[:, 0:1], rm[:, 0:1])

                # broadcast rstd/m_rs across R partitions:
                # rm^T via PE, then E^T @ rm_row -> [64, 128]
                rm_ps = ps_bc.tile([2, P], F32, tag="rm")
                nc.tensor.transpose(rm_ps[:], rm[:], id_sb[:])
                rm_row = spool.tile([2, P], F32)
                nc.scalar.copy(rm_row[:], rm_ps[:])
                bc_ps = ps_bc.tile([2 * R, P], F32, tag="bc")
                nc.tensor.matmul(bc_ps[:], e_sb[:], rm_row[:], start=True, stop=True)
                bc_sb = spool.tile([2 * R, P], F32)
                nc.scalar.copy(bc_sb[:], bc_ps[:])

                # ---- PE transpose of x chunks; ACT copies PSUM->SBUF ----
                xt_sb = xtpool.tile([P, D], F32)
                for c in range(N_CHUNK):
                    cs = slice(c * P, (c + 1) * P)
                    xt_ps = ps_xt.tile([P, P], F32)
                    nc.tensor.transpose(xt_ps[:], x_t[:, cs], id_sb[:])
                    nc.scalar.copy(xt_sb[:, cs], xt_ps[:])

                # ---- down-proj: accumulate W_c^T @ xT_c -> [R, 128 tok] ----
                dn_ps = ps_dn.tile([R, P], F32)
                for c in range(N_CHUNK):
                    cs = slice(c * P, (c + 1) * P)
                    nc.tensor.matmul(
                        dn_ps[:], w_sb[:, c, :], xt_sb[:, cs],
                        start=(c == 0), stop=(c == N_CHUNK - 1),
                    )

                # ---- LN correction + bias, in [R, 128] layout ----
                # down_true^T = rstd_b * dn - mrs_b * S + b2
                t1 = spool.tile([R, P], F32, tag="t1")
                nc.vector.tensor_mul(t1[:], dn_ps[:], bc_sb[0:R, :])
                t2 = spool.tile([R, P], F32, tag="t2")
                nc.vector.tensor_scalar(t2[:], bc_sb[R:2 * R, :], s_sb[:], None, ALU.mult)
                gin = spool.tile([R, P], F32, tag="gin")
                nc.vector.tensor_sub(gin[:], t1[:], t2[:])

                # ---- GELU (exact) with b2 as per-partition bias; ones row ----
                gt_sb = spool.tile([R + 1, P], F32, tag="gt")
                nc.scalar.activation(gt_sb[0:R, :], gin[:], AF.Gelu, bias=b2_sb[:])
                nc.vector.memset(gt_sb[R:R + 1, :], 1.0)

                # ---- up-proj (+b_up via ones row) and residual add ----
                out_t = opool.tile([P, D], F32)
                for j in range(N_UP):
                    js = slice(j * UP_N, (j + 1) * UP_N)
                    up_ps = ps_up.tile([P, UP_N], F32)
                    nc.tensor.matmul(up_ps[:], gt_sb[:], wu_sb[:, js], start=True, stop=True)
                    nc.vector.tensor_add(out_t[:, js], x_t[:, js], up_ps[:])

                nc.sync.dma_start(out_d[tok, :], out_t[:])

    nc.compile()
    return nc


def make_param_maps(gamma, beta, w_down, b_down, w_up, b_up):
    f32 = np.float32
    gamma = np.asarray(gamma, f32)
    beta = np.asarray(beta, f32)
    w_down = np.asarray(w_down, f32)
    b_down = np.asarray(b_down, f32)
    w_up = np.asarray(w_up, f32)
    b_up = np.asarray(b_up, f32)

    W = (gamma[:, None] * w_down).astype(f32)                   # [D, R]
    wc = np.ascontiguousarray(W.reshape(N_CHUNK, P, R).transpose(1, 0, 2))  # [P, c, R]
    s_col = W.sum(axis=0, dtype=f32).reshape(R, 1).astype(f32)
    b2_col = (beta @ w_down + b_down).astype(f32).reshape(R, 1)
    wu = np.concatenate([w_up, b_up[None, :]], axis=0).astype(f32)  # [R+1, D]
    e_mat = np.zeros((2, 2 * R), f32)
    e_mat[0, 0:R] = 1.0
    e_mat[1, R:2 * R] = 1.0
    ident = np.eye(P, dtype=f32)
    return {
        "wc": wc, "wu": wu, "s_col": s_col, "b2_col": b2_col,
        "e_mat": e_mat, "ident": ident,
    }


_NC_CACHE = None


def _get_nc():
    global _NC_CACHE
    if _NC_CACHE is None:
        _NC_CACHE = build_program()
    return _NC_CACHE


LAST_RESULTS = None  # BassKernelResults from the most recent run (for test.py)


def kernel(x, gamma, beta, w_down, b_down, w_up, b_up, _trace=False):
    global LAST_RESULTS
    from concourse.bass_utils import run_bass_kernel_spmd

    x = np.asarray(x, np.float32)
    params = make_param_maps(gamma, beta, w_down, b_down, w_up, b_up)

    x_flat = x.reshape(TOK_TOTAL, D)
    in_maps = []
    for c in range(N_CORES):
        shard = np.ascontiguousarray(
            x_flat[c * TOK_PER_CORE:(c + 1) * TOK_PER_CORE]
        )
        in_maps.append({"x": shard, **params})

    nc = _get_nc()
    res = run_bass_kernel_spmd(nc, in_maps, list(range(N_CORES)))
    LAST_RESULTS = res
    out = np.concatenate([res.results[c]["out"] for c in range(N_CORES)], axis=0)
    return out.reshape(x.shape).astype(np.float32)


# revision 18
# speedup vs baseline: 2.4811x; 2.4811x over previous
"""Bottleneck adapter (LayerNorm -> down-proj -> GELU -> up-proj -> residual)
as a Bass/Tile kernel for Trainium2, data-parallel over 8 NeuronCores.

Math (per token t, d_model D=2048, rank R=32):
    mu    = mean(x_t);  var = mean(x_t^2) - mu^2;  rstd = 1/sqrt(var+eps)
    down  = ln(x_t) @ w_down + b_down
          = rstd * (x_t @ W - mu * S) + b2        # W = gamma[:,None]*w_down
                                                  # S = colsum(W), b2 = beta@w_down + b_down
    out_t = x_t + gelu(down) @ w_up + b_up

Implementation notes (all chosen from cost-model/HW measurements):
  - down/up matmuls in bf16 (fp32 matmul is quarter-rate on the PE); the
    residual path and all statistics stay fp32.
  - x is PE-transposed per 128x128 chunk (transpose-mode matmul), batched
    4-chunks-per-PSUM-bank, copied to SBUF with a bf16 cast split across
    ACT and DVE.
  - mean arrives free as a ones-column of the bf16 down matmul (row R of the
    [R+1, 128] PSUM accumulator); sumsq comes from one ACT Square pass with
    accum_out. Both are per-token f32.
  - rstd = rsqrt(var+eps) is computed on DVE with the int-bit-trick seed
    (0x5f3759df) + 2 Newton iterations -- avoids the ACT Sqrt function-table
    load that would otherwise thrash against Gelu every tile (~2.6us/tile).
  - Gelu (exact, erf-based LUT) is the only table-based ACT function used,
    so its table loads exactly once. Copy/Square/Identity are in every set.
  - b_up rides as a 33rd contraction row of the up matmul against a constant
    ones row in the (persistent, double-buffered) gelu output tiles.
"""

import numpy as np

import concourse.bacc as bacc
import concourse.bass as bass
import concourse.tile as tile
from concourse import mybir

F32 = mybir.dt.float32
BF16 = mybir.dt.bfloat16
I32 = mybir.dt.int32
AF = mybir.ActivationFunctionType
ALU = mybir.AluOpType

D = 2048          # d_model
R = 32            # adapter rank
N_CORES = 8
TOK_TOTAL = 4 * 4096
TOK_PER_CORE = TOK_TOTAL // N_CORES   # 2048
P = 128           # partitions / tokens per tile
N_TILES = TOK_PER_CORE // P           # 16
N_CHUNK = D // P                      # 16 chunks of d per tile
LN_EPS = 1e-5
UP_N = 512        # free-dim per up matmul (one PSUM bank)
N_UP = D // UP_N  # 4
XB = 4            # transpose chunks batched per PSUM bank ([128, 512])
MAGIC = 0x5F3759DF  # rsqrt seed


def build_program(reps=1):
    """reps>1 repeats the whole computation in one NEFF — used only by the
    timing harness (wall-clock slope over reps isolates on-device time)."""
    nc = bacc.Bacc(
        "TRN2",
        target_bir_lowering=False,
        debug=False,
        num_devices=N_CORES,
    )

    x_d = nc.dram_tensor("x", [TOK_PER_CORE, D], F32, kind="ExternalInput").ap()
    w_d = nc.dram_tensor("wc", [P, N_CHUNK, R + 1], BF16, kind="ExternalInput").ap()
    wu_d = nc.dram_tensor("wu", [R + 1, D], BF16, kind="ExternalInput").ap()
    s_d = nc.dram_tensor("s_col", [R, 1], F32, kind="ExternalInput").ap()
    b2_d = nc.dram_tensor("b2_col", [R, 1], F32, kind="ExternalInput").ap()
    e_d = nc.dram_tensor("e_mat", [2, 2 * R], F32, kind="ExternalInput").ap()
    id_d = nc.dram_tensor("ident", [P, P], F32, kind="ExternalInput").ap()
    out_d = nc.dram_tensor("out", [TOK_PER_CORE, D], F32, kind="ExternalOutput").ap()

    with tile.TileContext(nc) as tc:
        with (
            tc.tile_pool(name="consts", bufs=1) as cpool,
            tc.tile_pool(name="xin", bufs=6) as xpool,
            tc.tile_pool(name="sq", bufs=2) as sqpool,
            tc.tile_pool(name="ssqp", bufs=4) as ssqpool,
            tc.tile_pool(name="xt", bufs=3) as xtpool,
            tc.tile_pool(name="outs", bufs=2) as opool,
            tc.tile_pool(name="small", bufs=2) as spool,
            tc.tile_pool(name="ps_xt", bufs=2, space="PSUM") as ps_xt,
            tc.tile_pool(name="ps_dn", bufs=3, space="PSUM") as ps_dn,
            tc.tile_pool(name="ps_sm", bufs=2, space="PSUM") as ps_sm,
            tc.tile_pool(name="ps_up", bufs=1, space="PSUM") as ps_up,
        ):
            # ---- one-time constant loads / setup ----
            w_sb = cpool.tile([P, N_CHUNK, R + 1], BF16)  # [W | ones] chunks
            nc.sync.dma_start(w_sb[:], w_d[:])
            wu_sb = cpool.tile([R + 1, D], BF16)          # [w_up; b_up]
            nc.sync.dma_start(wu_sb[:], wu_d[:])
            s_sb = cpool.tile([R, 1], F32)                # colsum(W)
            nc.sync.dma_start(s_sb[:], s_d[:])
            b2_sb = cpool.tile([R, 1], F32)               # beta@w_down + b_down
            nc.sync.dma_start(b2_sb[:], b2_d[:])
            e_sb = cpool.tile([2, 2 * R], F32)            # broadcast selector
            nc.sync.dma_start(e_sb[:], e_d[:])
            id_sb = cpool.tile([P, P], F32)               # identity for PE transpose
            nc.sync.dma_start(id_sb[:], id_d[:])
            magic_sb = cpool.tile([P, 1], I32)            # rsqrt seed constant
            nc.vector.memset(magic_sb[:], MAGIC)
            # persistent gelu-output tiles; row R is the ones-row for b_up
            gts = [cpool.tile([R + 1, P], BF16, tag=f"gt{j}", name=f"gt{j}")
                   for j in range(3)]
            for g in gts:
                nc.vector.memset(g[R:R + 1, :], 1.0)

            # Per-tile state passed between pipeline stages
            state = {}

            def stage_front(i):
                """Load + sumsq + PE transposes + bf16 down matmuls."""
                ti = i % N_TILES
                tok = slice(ti * P, (ti + 1) * P)
                x_t = xpool.tile([P, D], F32, tag="x", name=f"x_{i}")
                nc.sync.dma_start(x_t[:], x_d[tok, :])

                sq_scr = sqpool.tile([P, D], F32, tag="scr", name=f"sq_{i}")
                ssq = ssqpool.tile([P, 1], F32, tag="ssq", name=f"ssq_{i}")
                nc.scalar.activation(sq_scr[:], x_t[:], AF.Square, accum_out=ssq[:])

                xt_sb = xtpool.tile([P, D], BF16, tag="xt", name=f"xt_{i}")
                for b in range(N_CHUNK // XB):
                    xt_ps = ps_xt.tile([P, XB * P], F32, tag="xtps", name=f"xtps_{i}_{b}")
                    for c in range(XB):
                        nc.tensor.transpose(
                            xt_ps[:, c * P:(c + 1) * P],
                            x_t[:, (b * XB + c) * P:(b * XB + c + 1) * P],
                            id_sb[:],
                        )
                    dst = xt_sb[:, b * XB * P:(b + 1) * XB * P]
                    if b % 2 == 0:
                        nc.scalar.copy(dst, xt_ps[:])         # ACT, casts to bf16
                    else:
                        nc.vector.tensor_copy(dst, xt_ps[:])  # DVE, casts to bf16

                dn_ps = ps_dn.tile([R + 1, P], F32, tag="dn", name=f"dn_{i}")
                for c in range(N_CHUNK):
                    nc.tensor.matmul(
                        dn_ps[:], w_sb[:, c, :], xt_sb[:, c * P:(c + 1) * P],
                        start=(c == 0), stop=(c == N_CHUNK - 1),
                    )
                state[i] = {"x_t": x_t, "ssq": ssq, "dn_ps": dn_ps}

            def stage_mid(i):
                """Stats -> rstd (Newton on DVE) -> broadcast -> corr -> GELU."""
                st = state[i]
                ssq, dn_ps = st["ssq"], st["dn_ps"]

                s1_row = spool.tile([1, P], F32, tag="s1r", name=f"s1r_{i}")
                nc.scalar.copy(s1_row[:], dn_ps[R:R + 1, :])   # sum_d x (=D*mu)
                s1_ps = ps_sm.tile([P, 1], F32, tag="sm", name=f"s1ps_{i}")
                nc.tensor.transpose(s1_ps[:], s1_row[:], id_sb[0:1, 0:1])

                m_col = spool.tile([P, 1], F32, tag="mcol", name=f"m_{i}")
                nc.vector.tensor_scalar(m_col[:], s1_ps[:], 1.0 / D, None, ALU.mult)
                v0 = spool.tile([P, 1], F32, tag="v0", name=f"v0_{i}")
                nc.vector.tensor_scalar(v0[:], ssq[:], 1.0 / D, LN_EPS,
                                        ALU.mult, ALU.add)
                musq = spool.tile([P, 1], F32, tag="musq", name=f"musq_{i}")
                nc.vector.tensor_mul(musq[:], m_col[:], m_col[:])
                v = spool.tile([P, 1], F32, tag="v", name=f"v_{i}")
                nc.vector.tensor_sub(v[:], v0[:], musq[:])
                # seed y0 = bits(magic - (bits(v) >> 1)); 2 Newton iterations
                yi = spool.tile([P, 1], I32, tag="yi", name=f"yi_{i}")
                nc.vector.tensor_scalar(yi[:], v[:].bitcast(I32), 1, None,
                                        ALU.logical_shift_right)
                nc.vector.tensor_sub(yi[:], magic_sb[:], yi[:])
                y = yi[:].bitcast(F32)
                rm = spool.tile([P, 2], F32, tag="rm", name=f"rm_{i}")
                t1 = spool.tile([P, 1], F32, tag="nt1", name=f"nt1_{i}")
                for it_n in range(2):
                    nc.vector.tensor_mul(t1[:], y, y)
                    nc.vector.tensor_mul(t1[:], t1[:], v[:])
                    nc.vector.tensor_scalar(t1[:], t1[:], -0.5, 1.5,
                                            ALU.mult, ALU.add)
                    if it_n == 0:
                        nc.vector.tensor_mul(yi[:].bitcast(F32), y, t1[:])
                    else:
                        nc.vector.tensor_mul(rm[:, 0:1], y, t1[:])
                nc.vector.tensor_mul(rm[:, 1:2], m_col[:], rm[:, 0:1])

                rm_ps = ps_sm.tile([2, P], F32, tag="sm", name=f"rmps_{i}")
                nc.tensor.transpose(rm_ps[:], rm[:], id_sb[:])
                rm_row = spool.tile([2, P], F32, tag="rmrow", name=f"rmrow_{i}")
                nc.scalar.copy(rm_row[:], rm_ps[:])
                bc_ps = ps_sm.tile([2 * R, P], F32, tag="sm", name=f"bcps_{i}")
                nc.tensor.matmul(bc_ps[:], e_sb[:], rm_row[:], start=True, stop=True)
                bc_sb = spool.tile([2 * R, P], F32, tag="bc", name=f"bc_{i}")
                nc.scalar.copy(bc_sb[:], bc_ps[:])

                o1 = spool.tile([R, P], F32, tag="o1", name=f"o1_{i}")
                nc.vector.tensor_mul(o1[:], dn_ps[0:R, :], bc_sb[0:R, :])
                o2 = spool.tile([R, P], F32, tag="o2", name=f"o2_{i}")
                nc.vector.tensor_scalar(o2[:], bc_sb[R:2 * R, :], s_sb[:],
                                        None, ALU.mult)
                gin = spool.tile([R, P], F32, tag="gin", name=f"gin_{i}")
                nc.vector.tensor_sub(gin[:], o1[:], o2[:])

                gt_sb = gts[i % 3]
                nc.scalar.activation(gt_sb[0:R, :], gin[:], AF.Gelu, bias=b2_sb[:])

            def stage_back(i):
                """bf16 up-proj + residual + store."""
                ti = i % N_TILES
                tok = slice(ti * P, (ti + 1) * P)
                x_t = state[i]["x_t"]
                gt_sb = gts[i % 3]
                out_t = opool.tile([P, D], F32, tag="out", name=f"out_{i}")
                for j in range(N_UP):
                    js = slice(j * UP_N, (j + 1) * UP_N)
                    up_ps = ps_up.tile([P, UP_N], F32, tag="up", name=f"up_{i}_{j}")
                    nc.tensor.matmul(up_ps[:], gt_sb[:], wu_sb[:, js],
                                     start=True, stop=True)
                    nc.vector.tensor_add(out_t[:, js], x_t[:, js], up_ps[:])
                nc.scalar.dma_start(out_d[tok, :], out_t[:])
                del state[i]

            # 3-stage software pipeline: F(i) | M(i-1) | B(i-2). Keeps the PE
            # fed with tile i's transposes/matmuls while tile i-1's LN-stats
            # chain crosses engines.
            n_it = N_TILES * reps
            for it in range(n_it + 3):
                if it < n_it:
                    stage_front(it)
                if 0 <= it - 2 < n_it:
                    stage_mid(it - 2)
                if 0 <= it - 3 < n_it:
                    stage_back(it - 3)

    nc.compile()
    return nc


def make_param_maps(gamma, beta, w_down, b_down, w_up, b_up):
    import ml_dtypes

    f32 = np.float32
    bf16 = ml_dtypes.bfloat16
    gamma = np.asarray(gamma, f32)
    beta = np.asarray(beta, f32)
    w_down = np.asarray(w_down, f32)
    b_down = np.asarray(b_down, f32)
    w_up = np.asarray(w_up, f32)
    b_up = np.asarray(b_up, f32)

    W = (gamma[:, None] * w_down).astype(f32)                    # [D, R]
    W_bf = W.astype(bf16)
    w_aug = np.concatenate([W_bf, np.ones((D, 1), bf16)], axis=1)  # [D, R+1]
    wc = np.ascontiguousarray(
        w_aug.reshape(N_CHUNK, P, R + 1).transpose(1, 0, 2))       # [P, c, R+1]
    # S must match the bf16 W actually used in the matmul
    s_col = W_bf.astype(f32).sum(axis=0).reshape(R, 1).astype(f32)
    b2_col = (beta @ w_down + b_down).astype(f32).reshape(R, 1)
    wu = np.concatenate([w_up, b_up[None, :]], axis=0).astype(bf16)  # [R+1, D]
    e_mat = np.zeros((2, 2 * R), f32)
    e_mat[0, 0:R] = 1.0
    e_mat[1, R:2 * R] = 1.0
    ident = np.eye(P, dtype=f32)
    return {
        "wc": wc, "wu": wu, "s_col": s_col, "b2_col": b2_col,
        "e_mat": e_mat, "ident": ident,
    }


_NC_CACHE = None


def _get_nc():
    global _NC_CACHE
    if _NC_CACHE is None:
        _NC_CACHE = build_program()
    return _NC_CACHE


LAST_RESULTS = None  # BassKernelResults from the most recent run (for test.py)


def kernel(x, gamma, beta, w_down, b_down, w_up, b_up):
    global LAST_RESULTS
    from concourse.bass_utils import run_bass_kernel_spmd

    x = np.asarray(x, np.float32)
    params = make_param_maps(gamma, beta, w_down, b_down, w_up, b_up)

    x_flat = x.reshape(TOK_TOTAL, D)
    in_maps = []
    for c in range(N_CORES):
        shard = np.ascontiguousarray(
            x_flat[c * TOK_PER_CORE:(c + 1) * TOK_PER_CORE]
        )
        in_maps.append({"x": shard, **params})

    nc = _get_nc()
    res = run_bass_kernel_spmd(nc, in_maps, list(range(N_CORES)))
    LAST_RESULTS = res
    out = np.concatenate([res.results[c]["out"] for c in range(N_CORES)], axis=0)
    return out.reshape(x.shape).astype(np.float32)


# revision 21
# speedup vs baseline: 2.4895x; 1.0034x over previous
"""Bottleneck adapter (LayerNorm -> down-proj -> GELU -> up-proj -> residual)
as a Bass/Tile kernel for Trainium2, data-parallel over 8 NeuronCores.

Math (per token t, d_model D=2048, rank R=32):
    mu    = mean(x_t);  var = mean(x_t^2) - mu^2;  rstd = 1/sqrt(var+eps)
    down  = ln(x_t) @ w_down + b_down
          = rstd * (x_t @ W - mu * S) + b2        # W = gamma[:,None]*w_down
                                                  # S = colsum(W), b2 = beta@w_down + b_down
    out_t = x_t + gelu(down) @ w_up + b_up

Implementation notes (from cost-model + HW slope measurements):
  - down/up matmuls in bf16 (fp32 matmul is quarter-rate on the PE); the
    residual path and all statistics stay fp32.
  - x is PE-transposed per 128x128 chunk (transpose-mode matmul), batched
    4-chunks-per-PSUM-bank, copied to SBUF with a bf16 cast split across
    ACT and DVE.
  - The down matmul keeps TOKENS on the output partition axis
    (lhsT = xT chunk, rhs = [W | ones]), so mean (the ones-column) and all
    LN statistics are per-partition scalars: the correction needs no
    partition-broadcast at all, just tensor_scalar/scalar_tensor_tensor.
  - rstd = rsqrt(var+eps) on DVE with the int-bit-trick seed (0x5f3759df)
    + 2 Newton iterations -- avoids the ACT Sqrt function-table load that
    would thrash against Gelu every tile.
  - Gelu (exact, erf-based LUT) is the only table-based ACT function used,
    so its table loads exactly once. Copy/Square/Identity are in every set.
  - b_up rides as a 33rd contraction row of the up matmul against constant
    ones rows in persistent gelu-output tiles.
  - 4-stage software pipeline (front / -- / mid / back) keeps the PE fed
    with tile i's transposes+matmuls while tile i-2's stats chain crosses
    engines; x loads ride the SP HWDGE ring, stores the ACT ring.
"""

import numpy as np

import concourse.bacc as bacc
import concourse.bass as bass
import concourse.tile as tile
from concourse import mybir

F32 = mybir.dt.float32
BF16 = mybir.dt.bfloat16
I32 = mybir.dt.int32
AF = mybir.ActivationFunctionType
ALU = mybir.AluOpType

D = 2048          # d_model
R = 32            # adapter rank
N_CORES = 8
TOK_TOTAL = 4 * 4096
TOK_PER_CORE = TOK_TOTAL // N_CORES   # 2048
P = 128           # partitions / tokens per tile
N_TILES = TOK_PER_CORE // P           # 16
N_CHUNK = D // P                      # 16 chunks of d per tile
LN_EPS = 1e-5
UP_N = 512        # free-dim per up matmul (one PSUM bank)
N_UP = D // UP_N  # 4
XB = 4            # transpose chunks batched per PSUM bank ([128, 512])
MAGIC = 0x5F3759DF  # rsqrt seed


def build_program(reps=1):
    """reps>1 repeats the whole computation in one NEFF — used only by the
    timing harness (wall-clock slope over reps isolates on-device time)."""
    nc = bacc.Bacc(
        "TRN2",
        target_bir_lowering=False,
        debug=False,
        num_devices=N_CORES,
    )

    x_d = nc.dram_tensor("x", [TOK_PER_CORE, D], F32, kind="ExternalInput").ap()
    w_d = nc.dram_tensor("wc", [P, N_CHUNK, R + 1], BF16, kind="ExternalInput").ap()
    wu_d = nc.dram_tensor("wu", [R + 1, D], BF16, kind="ExternalInput").ap()
    sd_d = nc.dram_tensor("sd_bc", [P, R], F32, kind="ExternalInput").ap()
    b2_d = nc.dram_tensor("b2_bc", [P, R], F32, kind="ExternalInput").ap()
    id_d = nc.dram_tensor("ident", [P, P], F32, kind="ExternalInput").ap()
    out_d = nc.dram_tensor("out", [TOK_PER_CORE, D], F32, kind="ExternalOutput").ap()

    with tile.TileContext(nc) as tc:
        with (
            tc.tile_pool(name="consts", bufs=1) as cpool,
            tc.tile_pool(name="xin", bufs=6) as xpool,
            tc.tile_pool(name="sq", bufs=2) as sqpool,
            tc.tile_pool(name="ssqp", bufs=4) as ssqpool,
            tc.tile_pool(name="xt", bufs=3) as xtpool,
            tc.tile_pool(name="outs", bufs=2) as opool,
            tc.tile_pool(name="small", bufs=2) as spool,
            tc.tile_pool(name="ps_xt", bufs=2, space="PSUM") as ps_xt,
            tc.tile_pool(name="ps_dn", bufs=3, space="PSUM") as ps_dn,
            tc.tile_pool(name="ps_gt", bufs=2, space="PSUM") as ps_gt,
            tc.tile_pool(name="ps_up", bufs=1, space="PSUM") as ps_up,
        ):
            # ---- one-time constant loads / setup ----
            w_sb = cpool.tile([P, N_CHUNK, R + 1], BF16)  # [W | ones] chunks
            nc.sync.dma_start(w_sb[:], w_d[:])
            wu_sb = cpool.tile([R + 1, D], BF16)          # [w_up; b_up]
            nc.sync.dma_start(wu_sb[:], wu_d[:])
            sd_sb = cpool.tile([P, R], F32)               # colsum(W)/D, bcast
            nc.sync.dma_start(sd_sb[:], sd_d[:])
            b2_sb = cpool.tile([P, R], F32)               # beta@w_down+b_down, bcast
            nc.sync.dma_start(b2_sb[:], b2_d[:])
            id_sb = cpool.tile([P, P], F32)               # identity for PE transpose
            nc.sync.dma_start(id_sb[:], id_d[:])
            magic_sb = cpool.tile([P, 1], I32)            # rsqrt seed constant
            nc.vector.memset(magic_sb[:], MAGIC)
            # persistent gelu-output tiles; row R is the ones-row for b_up
            gts = [cpool.tile([R + 1, P], BF16, tag=f"gt{j}", name=f"gt{j}")
                   for j in range(3)]
            for g in gts:
                nc.vector.memset(g[R:R + 1, :], 1.0)

            # Per-tile state passed between pipeline stages
            state = {}

            def stage_front(i):
                """Load + sumsq + PE transposes + bf16 down matmuls."""
                ti = i % N_TILES
                tok = slice(ti * P, (ti + 1) * P)
                x_t = xpool.tile([P, D], F32, tag="x", name=f"x_{i}")
                nc.sync.dma_start(x_t[:], x_d[tok, :])

                sq_scr = sqpool.tile([P, D], F32, tag="scr", name=f"sq_{i}")
                ssq = ssqpool.tile([P, 1], F32, tag="ssq", name=f"ssq_{i}")
                nc.scalar.activation(sq_scr[:], x_t[:], AF.Square, accum_out=ssq[:])

                xt_sb = xtpool.tile([P, D], BF16, tag="xt", name=f"xt_{i}")
                for b in range(N_CHUNK // XB):
                    xt_ps = ps_xt.tile([P, XB * P], F32, tag="xtps",
                                       name=f"xtps_{i}_{b}")
                    for c in range(XB):
                        nc.tensor.transpose(
                            xt_ps[:, c * P:(c + 1) * P],
                            x_t[:, (b * XB + c) * P:(b * XB + c + 1) * P],
                            id_sb[:],
                        )
                    dst = xt_sb[:, b * XB * P:(b + 1) * XB * P]
                    if b % 2 == 0:
                        nc.scalar.copy(dst, xt_ps[:])         # ACT, casts to bf16
                    else:
                        nc.vector.tensor_copy(dst, xt_ps[:])  # DVE, casts to bf16

                # down-proj, tokens on partitions: xT_c^T @ [W_c | 1]
                dn_ps = ps_dn.tile([P, R + 1], F32, tag="dn", name=f"dn_{i}")
                for c in range(N_CHUNK):
                    nc.tensor.matmul(
                        dn_ps[:], xt_sb[:, c * P:(c + 1) * P], w_sb[:, c, :],
                        start=(c == 0), stop=(c == N_CHUNK - 1),
                    )
                state[i] = {"x_t": x_t, "ssq": ssq, "dn_ps": dn_ps}

            def stage_mid(i):
                """LN stats -> rstd (Newton, DVE) -> correction -> GELU -> g^T."""
                st = state[i]
                ssq, dn_ps = st["ssq"], st["dn_ps"]
                s1 = dn_ps[:, R:R + 1]                      # sum_d x  (= D*mu)

                # var = (ssq - s1^2/D)/D ; all per-partition [128,1] f32
                # (s1 lives in PSUM; DVE has a single PSUM read port, so pull
                # it into SBUF before squaring it)
                s1_sb = spool.tile([P, 1], F32, tag="s1", name=f"s1_{i}")
                nc.vector.tensor_scalar(s1_sb[:], s1, 1.0, None, ALU.mult)
                p_t = spool.tile([P, 1], F32, tag="p", name=f"p_{i}")
                nc.vector.tensor_mul(p_t[:], s1_sb[:], s1_sb[:])
                v = spool.tile([P, 1], F32, tag="v", name=f"v_{i}")
                nc.vector.scalar_tensor_tensor(v[:], p_t[:], -1.0 / D, ssq[:],
                                               ALU.mult, ALU.add)
                nc.vector.tensor_scalar(v[:], v[:], 1.0 / D, LN_EPS,
                                        ALU.mult, ALU.add)
                # rstd = rsqrt(v): bit-trick seed + 2 Newton iterations
                yi = spool.tile([P, 1], I32, tag="yi", name=f"yi_{i}")
                nc.vector.tensor_scalar(yi[:], v[:].bitcast(I32), 1, None,
                                        ALU.logical_shift_right)
                nc.vector.tensor_sub(yi[:], magic_sb[:], yi[:])
                y = yi[:].bitcast(F32)
                rstd = spool.tile([P, 1], F32, tag="rstd", name=f"rstd_{i}")
                t1 = spool.tile([P, 1], F32, tag="nt1", name=f"nt1_{i}")
                for it_n in range(2):
                    nc.vector.tensor_mul(t1[:], y, y)
                    nc.vector.tensor_mul(t1[:], t1[:], v[:])
                    nc.vector.tensor_scalar(t1[:], t1[:], -0.5, 1.5,
                                            ALU.mult, ALU.add)
                    if it_n == 0:
                        nc.vector.tensor_mul(yi[:].bitcast(F32), y, t1[:])
                    else:
                        nc.vector.tensor_mul(rstd[:], y, t1[:])
                mrs = spool.tile([P, 1], F32, tag="mrs", name=f"mrs_{i}")
                nc.vector.tensor_mul(mrs[:], s1_sb[:], rstd[:])   # = D*mu*rstd

                # o2 = (S/D)*mrs - b2 ; gin = rstd*down_raw - o2
                o2 = spool.tile([P, R], F32, tag="o2", name=f"o2_{i}")
                nc.vector.scalar_tensor_tensor(o2[:], sd_sb[:], mrs[:], b2_sb[:],
                                               ALU.mult, ALU.subtract)
                gin = spool.tile([P, R], F32, tag="gin", name=f"gin_{i}")
                nc.vector.scalar_tensor_tensor(gin[:], dn_ps[:, 0:R], rstd[:],
                                               o2[:], ALU.mult, ALU.subtract)

                # exact GELU, then transpose g -> [R, 128] for the up matmul
                g_t = spool.tile([P, R], F32, tag="g", name=f"g_{i}")
                nc.scalar.activation(g_t[:], gin[:], AF.Gelu)
                gt_ps = ps_gt.tile([R, P], F32, tag="gt", name=f"gtps_{i}")
                nc.tensor.transpose(gt_ps[:], g_t[:], id_sb[:])
                nc.scalar.copy(gts[i % 3][0:R, :], gt_ps[:])  # casts to bf16

            def stage_back(i):
                """bf16 up-proj + residual + store."""
                ti = i % N_TILES
                tok = slice(ti * P, (ti + 1) * P)
                x_t = state[i]["x_t"]
                gt_sb = gts[i % 3]
                out_t = opool.tile([P, D], F32, tag="out", name=f"out_{i}")
                for j in range(N_UP):
                    js = slice(j * UP_N, (j + 1) * UP_N)
                    up_ps = ps_up.tile([P, UP_N], F32, tag="up", name=f"up_{i}_{j}")
                    nc.tensor.matmul(up_ps[:], gt_sb[:], wu_sb[:, js],
                                     start=True, stop=True)
                    nc.vector.tensor_add(out_t[:, js], x_t[:, js], up_ps[:])
                nc.scalar.dma_start(out_d[tok, :], out_t[:])
                del state[i]

            # 4-stage software pipeline: F(i) | - | M(i-2) | B(i-3).
            n_it = N_TILES * reps
            for it in range(n_it + 3):
                if it < n_it:
                    stage_front(it)
                if 0 <= it - 2 < n_it:
                    stage_mid(it - 2)
                if 0 <= it - 3 < n_it:
                    stage_back(it - 3)

    nc.compile()
    return nc


def make_param_maps(gamma, beta, w_down, b_down, w_up, b_up):
    import ml_dtypes

    f32 = np.float32
    bf16 = ml_dtypes.bfloat16
    gamma = np.asarray(gamma, f32)
    beta = np.asarray(beta, f32)
    w_down = np.asarray(w_down, f32)
    b_down = np.asarray(b_down, f32)
    w_up = np.asarray(w_up, f32)
    b_up = np.asarray(b_up, f32)

    W = (gamma[:, None] * w_down).astype(f32)                    # [D, R]
    W_bf = W.astype(bf16)
    w_aug = np.concatenate([W_bf, np.ones((D, 1), bf16)], axis=1)  # [D, R+1]
    wc = np.ascontiguousarray(
        w_aug.reshape(N_CHUNK, P, R + 1).transpose(1, 0, 2))       # [P, c, R+1]
    # S must match the bf16 W actually used in the matmul; fold in the 1/D
    S = W_bf.astype(f32).sum(axis=0)
    sd_bc = np.tile((S / D).astype(f32)[None, :], (P, 1))
    b2 = (beta @ w_down + b_down).astype(f32)
    b2_bc = np.tile(b2[None, :], (P, 1))
    wu = np.concatenate([w_up, b_up[None, :]], axis=0).astype(bf16)  # [R+1, D]
    ident = np.eye(P, dtype=f32)
    return {
        "wc": wc, "wu": wu, "sd_bc": sd_bc, "b2_bc": b2_bc, "ident": ident,
    }


_NC_CACHE = None


def _get_nc():
    global _NC_CACHE
    if _NC_CACHE is None:
        _NC_CACHE = build_program()
    return _NC_CACHE


LAST_RESULTS = None  # BassKernelResults from the most recent run (for test.py)


def kernel(x, gamma, beta, w_down, b_down, w_up, b_up):
    global LAST_RESULTS
    from concourse.bass_utils import run_bass_kernel_spmd

    x = np.asarray(x, np.float32)
    params = make_param_maps(gamma, beta, w_down, b_down, w_up, b_up)

    x_flat = x.reshape(TOK_TOTAL, D)
    in_maps = []
    for c in range(N_CORES):
        shard = np.ascontiguousarray(
            x_flat[c * TOK_PER_CORE:(c + 1) * TOK_PER_CORE]
        )
        in_maps.append({"x": shard, **params})

    nc = _get_nc()
    res = run_bass_kernel_spmd(nc, in_maps, list(range(N_CORES)))
    LAST_RESULTS = res
    out = np.concatenate([res.results[c]["out"] for c in range(N_CORES)], axis=0)
    return out.reshape(x.shape).astype(np.float32)
